# revision 1
# baseline (speedup 1.0000x reference)
"""ChannelAttention (B,D,H,W,C = 4,8,32,32,512; 8 heads, head_dim 64) on 8
Trainium2 NeuronCores, Bass/Tile SPMD. Fully data-parallel: zero cross-core
communication.

Sharding: the 32768 tokens (B * D*H*W) are split 8 ways -> 4096 output tokens
per core; cores (2j, 2j+1) handle the two halves of batch j. Channel
attention needs the per-head 64x64 k^T v Gram matrix over ALL of a batch's
tokens, so each core redundantly computes k|v for its whole batch (8192
tokens; its own half ordered first in its xT input). This duplicated k|v work
(~55us of PE) is cheaper and far more robust than any cross-core reduction
(a ncfw AllReduce costs ~70us fixed + a 67us start barrier).

Schedule per core:
  pass A   : stream xT chunks (16 = both halves), k|v = x @ Wkv^T (k scale
             folded in on host), accumulate per-head-pair k^T v into PSUM
             (head-pair x head-quad blocking so fp32r matmuls hit the N>=256
             full-rate mode). HAM warm-up keeper matmuls run during the
             initial DMA fill.
  softmax  : pack 8 64x64 blocks, rowwise softmax over e on [128, 4, 64]
             (DVE reduce/reciprocal, ACT exp) -- overlapped with
  pass B   : qT = Wq @ x^T for the core's own 4096 tokens (re-streams x).
  phase 2  : PE-transpose probs into block-diagonal pair lhsT, out = attnT @
             qT, proj y = out @ Wproj^T (+bias via DVE broadcast add),
             software-pipelined one chunk ahead.

Numerics: all matmuls in float32r (fp32 storage, reduced-precision PE
multiply, ~13-bit effective mantissa) with fp32 PSUM accumulation; softmax in
fp32. End-to-end L2 relative error vs the fp32 reference: ~1.0e-3.
"""

import os
import numpy as np
from contextlib import ExitStack

import concourse.bass as bass
import concourse.mybir as mybir
import concourse.tile as tile
from concourse import bacc
from concourse.bass_utils import run_bass_kernel_spmd
from concourse.masks import make_identity

B, D, H, W, C = 4, 8, 32, 32, 512
NUM_HEADS = 8
HEAD_DIM = C // NUM_HEADS
SCALE = HEAD_DIM ** -0.5
N_TOK = B * D * H * W
N_CORES = 8
N_LOC = N_TOK // N_CORES
CHUNK = 512
N_CHUNKS = N_LOC // CHUNK
TT = 128
T_PER_CHUNK = CHUNK // TT
N_CI = C // 128
N_PAIRS = NUM_HEADS // 2

f32 = mybir.dt.float32
f32r = mybir.dt.float32r

N_KEEP_START = 24
N_KEEP_MID = 16

_NC_CACHE = None


def build_nc():
    nc = bacc.Bacc(num_devices=N_CORES)

    xT = nc.declare_dram_parameter("xT", [C, 2 * N_LOC], f32r, isOutput=False)
    wq = nc.declare_dram_parameter("wq", [C, C], f32r, isOutput=False)
    wkv = nc.declare_dram_parameter("wkv", [C, 2 * C], f32r, isOutput=False)
    wp = nc.declare_dram_parameter("wp", [C, C], f32r, isOutput=False)
    bp = nc.declare_dram_parameter("bp", [1, C], f32r, isOutput=False)
    y = nc.declare_dram_parameter("y", [N_LOC, C], f32, isOutput=True)

    xT_v = xT.rearrange("(a p) n -> p a n", p=128)
    wq_v = wq.rearrange("(a p) f -> p a f", p=128)
    wkv_v = wkv.rearrange("(a p) f -> p a f", p=128)
    wp_v = wp.rearrange("(a p) f -> p a f", p=128)

    with tile.TileContext(nc) as tc, ExitStack() as ctx:
        const = ctx.enter_context(tc.tile_pool(name="const", bufs=1))
        persist = ctx.enter_context(tc.tile_pool(name="persist", bufs=1))
        sb = ctx.enter_context(tc.tile_pool(name="sb", bufs=2))
        kvp = ctx.enter_context(tc.tile_pool(name="kvp", bufs=4))

        wkv_sb = const.tile([128, N_CI, 2 * C], f32r)
        nc.sync.dma_start(wkv_sb[:], wkv_v[:])
        ones_f32 = const.tile([1, 128], f32)
        nc.vector.memset(ones_f32[:], 1.0)
        ones_sb = const.tile([1, 128], f32r)
        nc.vector.tensor_copy(ones_sb[:], ones_f32[:])
        zrow_f32 = const.tile([1, 512], f32)
        nc.vector.memset(zrow_f32[:], 0.0)
        zrow_sb = const.tile([1, 512], f32r)
        nc.vector.tensor_copy(zrow_sb[:], zrow_f32[:])
        ident = const.tile([128, 128], f32)
        make_identity(nc, ident[:])

        qT_all = persist.tile([128, N_PAIRS, N_CHUNKS, CHUNK], f32r)
        red_sb = persist.tile([128, N_PAIRS, 64], f32)

        # ---------------- pass A: k|v + attn partial accumulation ----------
        with (
            tc.tile_pool(name="ps_kv", bufs=2, space="PSUM") as ps_kv,
            tc.tile_pool(name="ps_at", bufs=1, space="PSUM") as ps_at,
            tc.tile_pool(name="ps_q", bufs=2, space="PSUM") as ps_q,
        ):
            attn_ps = ps_at.tile([128, N_PAIRS, 256], f32)
            # bank-wide has_written seed + HAM warm-up
            for i in range(max(2, N_KEEP_START)):
                bank = i % 2
                nc.tensor.matmul(
                    attn_ps[:, 2 * bank:2 * bank + 2, :].rearrange("p a e -> p (a e)"),
                    ones_sb[:], zrow_sb[:],
                    start=(i < 2), stop=False, skip_group_check=True,
                )

            for c in range(2 * N_CHUNKS):
                xt = sb.tile([128, N_CI, CHUNK], f32r, tag="xt")
                nc.sync.dma_start(xt[:], xT_v[:, :, c * CHUNK:(c + 1) * CHUNK])

                kv_tiles = []
                for s in range(T_PER_CHUNK):
                    kv_ps = ps_kv.tile([128, 2 * C], f32, tag="kv")
                    for h in range(2):
                        for k in range(N_CI):
                            nc.tensor.matmul(
                                kv_ps[:, h * C:(h + 1) * C],
                                xt[:, k, s * TT:(s + 1) * TT],
                                wkv_sb[:, k, h * C:(h + 1) * C],
                                start=(k == 0), stop=(k == N_CI - 1),
                            )
                    kv_sb = kvp.tile([128, 2 * C], f32r, tag="kvsb")
                    nc.vector.tensor_copy(kv_sb[:], kv_ps[:])
                    kv_tiles.append(kv_sb)

                for s in range(T_PER_CHUNK):
                    kv_sb = kv_tiles[s]
                    for p in range(N_PAIRS):
                        q4 = p // 2
                        nc.tensor.matmul(
                            attn_ps[:, p, :],
                            kv_sb[:, p * 128:(p + 1) * 128],
                            kv_sb[:, C + q4 * 256:C + (q4 + 1) * 256],
                            start=False,
                            stop=(c == 2 * N_CHUNKS - 1 and s == T_PER_CHUNK - 1),
                            skip_group_check=True,
                        )

            # pack 8 useful 64x64 blocks -> red_sb[d + 64*(h%2), h//2, :]
            for h in range(NUM_HEADS):
                p = h // 2
                row0 = (h % 2) * 64
                col0 = (p % 2) * 128 + row0
                nc.vector.tensor_copy(
                    red_sb[row0:row0 + 64, h // 2, :],
                    attn_ps[row0:row0 + 64, p, col0:col0 + 64],
                )

            # weights for pass B / phase 2 (loaded once pass A's DMAs drain)
            wq_sb = const.tile([128, N_CI, C], f32r)
            nc.sync.dma_start(wq_sb[:], wq_v[:])
            wp_sb = const.tile([128, N_CI, C], f32r)
            nc.sync.dma_start(wp_sb[:], wp_v[:])
            bp_f32 = const.tile([128, C], f32)
            bp_bcast = bass.AP(
                tensor=bp[:].bitcast(f32).tensor,
                offset=0,
                ap=[[0, 128], [1, C]],
            )
            nc.sync.dma_start(bp_f32[:], bp_bcast)

            # ---- softmax over e on [128, pair, 64] (overlaps pass B) ----
            nmax = sb.tile([128, N_PAIRS, 1], f32, tag="nmax")
            nc.vector.reduce_max(nmax[:], red_sb[:], axis=mybir.AxisListType.X, negate=True)
            shifted = sb.tile([128, N_PAIRS, 64], f32, tag="shifted")
            nc.vector.tensor_add(shifted[:], red_sb[:], nmax.broadcast_to([128, N_PAIRS, 64]))
            expd = sb.tile([128, N_PAIRS, 64], f32, tag="expd")
            nc.scalar.activation(expd[:], shifted[:], mybir.ActivationFunctionType.Exp)
            ssum = sb.tile([128, N_PAIRS, 1], f32, tag="ssum")
            nc.vector.reduce_sum(ssum[:], expd[:], axis=mybir.AxisListType.X)
            rsum = sb.tile([128, N_PAIRS, 1], f32, tag="rsum")
            nc.vector.reciprocal(rsum[:], ssum[:])
            probs = sb.tile([128, N_PAIRS, 64], f32, tag="probs")
            nc.vector.tensor_mul(probs[:], expd[:], rsum.broadcast_to([128, N_PAIRS, 64]))
            probs2 = sb.tile([64, NUM_HEADS, 64], f32, tag="probs2")
            nc.vector.tensor_copy(probs2[:, 0::2, :], probs[0:64, :, :])
            nc.vector.tensor_copy(probs2[:, 1::2, :], probs[64:128, :, :])
            zro = sb.tile([128, N_PAIRS, 128], f32, tag="zro")
            nc.vector.memset(zro[:], 0.0)
            atnT = persist.tile([128, N_PAIRS, 128], f32r)
            nc.vector.tensor_copy(atnT[:], zro[:])

            # ------------- pass B: qT (overlaps the exchange) --------------
            for c in range(N_CHUNKS):
                xt = sb.tile([128, N_CI, CHUNK], f32r, tag="xtb")
                nc.sync.dma_start(xt[:], xT_v[:, :, c * CHUNK:(c + 1) * CHUNK])
                for p in range(N_PAIRS):
                    q_ps = ps_q.tile([128, CHUNK], f32, tag="q")
                    for k in range(N_CI):
                        nc.tensor.matmul(
                            q_ps[:],
                            wq_sb[:, k, p * 128:(p + 1) * 128],
                            xt[:, k, :],
                            start=(k == 0), stop=(k == N_CI - 1),
                        )
                    nc.scalar.copy(qT_all[:, p, c, :], q_ps[:])

        with (
            tc.tile_pool(name="ps_tr", bufs=1, space="PSUM") as ps_tr,
            tc.tile_pool(name="ps_keep", bufs=1, space="PSUM") as ps_keep,
            tc.tile_pool(name="ps_o", bufs=3, space="PSUM") as ps_o,
            tc.tile_pool(name="ps_y", bufs=3, space="PSUM") as ps_y,
        ):
            # HAM keepers in case the exchange outlasts pass B
            keep_ps = ps_keep.tile([128, C], f32)
            for i in range(N_KEEP_MID):
                nc.tensor.matmul(
                    keep_ps[:], ones_sb[:], zrow_sb[:],
                    start=(i == 0), stop=False, skip_group_check=True,
                )

            # ---- transpose probs -> block-diag pair lhsT (f32r) ----
            tr_ps = ps_tr.tile([64, NUM_HEADS, 64], f32)
            for h in range(NUM_HEADS):
                nc.tensor.transpose(tr_ps[:, h, :], probs2[:, h, :], ident[0:64, 0:64])
            for h in range(NUM_HEADS):
                p = h // 2
                off = (h % 2) * 64
                nc.vector.tensor_copy(
                    atnT[off:off + 64, p, off:off + 64], tr_ps[:, h, :]
                )

            # ---------------- phase 2: out + proj --------------------------
            def emit_out(c):
                outT_sb = sb.tile([128, N_CI, CHUNK], f32r, tag="outT", bufs=3, name=f"outT_{c}")
                for p in range(N_PAIRS):
                    o_ps = ps_o.tile([128, CHUNK], f32, tag="o", name=f"o_{c}_{p}")
                    nc.tensor.matmul(
                        o_ps[:], atnT[:, p, :], qT_all[:, p, c, :],
                        start=True, stop=True,
                    )
                    nc.scalar.copy(outT_sb[:, p, :], o_ps[:])
                return outT_sb

            outT_tiles = {0: emit_out(0), 1: emit_out(1)}
            for c in range(N_CHUNKS):
                if c + 2 < N_CHUNKS:
                    outT_tiles[c + 2] = emit_out(c + 2)
                outT_sb = outT_tiles.pop(c)
                for s in range(T_PER_CHUNK):
                    y_ps = ps_y.tile([128, C], f32, tag="y")
                    for k in range(N_CI):
                        nc.tensor.matmul(
                            y_ps[:],
                            outT_sb[:, k, s * TT:(s + 1) * TT],
                            wp_sb[:, k, :],
                            start=(k == 0), stop=(k == N_CI - 1),
                        )
                    y_sb = sb.tile([128, C], f32, tag="ysb", bufs=4)
                    nc.vector.tensor_add(y_sb[:], y_ps[:], bp_f32[:])
                    t0 = c * CHUNK + s * TT
                    nc.sync.dma_start(y[t0:t0 + TT, :], y_sb[:])

    nc.compile()
    return nc


def _get_nc():
    global _NC_CACHE
    if _NC_CACHE is None:
        _NC_CACHE = build_nc()
    return _NC_CACHE


def prep_inputs(x, Wqkv, Wproj, bproj):
    x = np.ascontiguousarray(np.asarray(x, dtype=np.float32))
    Wqkv = np.asarray(Wqkv, dtype=np.float32)
    Wproj = np.asarray(Wproj, dtype=np.float32)
    bproj = np.asarray(bproj, dtype=np.float32)

    xf = x.reshape(B, D * H * W, C)
    wq = np.ascontiguousarray(Wqkv[0:C].T)
    wk = Wqkv[C:2 * C] * np.float32(SCALE)
    wv = Wqkv[2 * C:3 * C]
    wkv = np.ascontiguousarray(np.concatenate([wk, wv], axis=0).T)
    wp = np.ascontiguousarray(Wproj.T)
    bp = np.ascontiguousarray(bproj.reshape(1, C))

    in_maps = []
    for i in range(N_CORES):
        b = i // 2
        t0 = (i % 2) * N_LOC
        own = xf[b, t0:t0 + N_LOC, :]
        pair = xf[b, N_LOC - t0:2 * N_LOC - t0, :]
        xTl = np.ascontiguousarray(np.concatenate([own, pair], axis=0).T)
        in_maps.append({"xT": xTl, "wq": wq, "wkv": wkv, "wp": wp, "bp": bp})
    return in_maps


def gather_output(results):
    parts = [np.asarray(results[i]["y"]) for i in range(N_CORES)]
    return np.concatenate(parts, axis=0).reshape(B, D, H, W, C)


def kernel(x, Wqkv, Wproj, bproj, _trace=False, _tmpdir=None):
    nc = _get_nc()
    in_maps = prep_inputs(x, Wqkv, Wproj, bproj)
    res = run_bass_kernel_spmd(
        nc, in_maps, list(range(N_CORES)), trace=_trace, tmpdir=_tmpdir
    )
    out = gather_output(res.results)
    if _trace:
        kernel.last_exec_time_ns = res.exec_time_ns
        kernel.last_results = res
    return out



# revision 8
# speedup vs baseline: 1.3661x; 1.3661x over previous
"""ChannelAttention (B,D,H,W,C = 4,8,32,32,512; 8 heads, head_dim 64) on 8
Trainium2 NeuronCores, Bass/Tile SPMD. Fully data-parallel, zero cross-core
communication, with an algebraic restructuring that collapses most of the
FLOPs:

  attn_h = softmax(SCALE * k_h^T v_h) where k = x Wk^T, v = x Wv^T.
  k_h^T v_h = Wk_h (x^T x) Wv_h^T, so the per-head 64x64 Gram matrices only
  need Gx = x^T x (512x512 over the batch's 8192 tokens). The output
  y = ((x Wq^T) BD^T) Wproj^T + b (BD = blockdiag(attn_h)) then folds into a
  single GEMM y = x Weff^T + b with Weff = Wproj BD Wq (512x512, per batch).

Sharding: cores (2j, 2j+1) handle the two 4096-token halves of batch j. Each
core computes Gx over the full batch (duplicated k/v-side work, far cheaper
than any cross-core reduction), the tiny weight-space chain, then the final
GEMM for its own 4096 tokens.

Schedule per core:
  pass A : stream x token-major (16 chunks of [128,4,512]), accumulate
           Gx into PSUM (4 accumulation groups of F=512 fp32r matmuls).
           HAM keeper matmuls cover the initial DMA fill.
  chain  : Gx -> A = Gx Wv^T -> Gram pairs (quad-blocked, F=256) ->
           rowwise softmax over e -> PE-transpose probs -> block-diag pair
           lhsT -> Wq_eff = BD Wq -> WeffT = Wq_eff^T Wproj^T (-> bf16).
  final  : y = x Weff^T + bias for own tokens, lhsT from a host-supplied
           bf16 x^T stream (DMAed during pass A), out streamed as bf16.

Numerics: fp32r matmuls with fp32 PSUM accumulation through the Gram/softmax
chain; the final GEMM runs in bf16 (error there is linear, no softmax
amplification) and y returns as bf16 -> f32. End-to-end L2 relative error vs
the fp32 reference: ~3e-3 (threshold 2e-2).
"""

import numpy as np
from contextlib import ExitStack

import ml_dtypes

import concourse.bass as bass
import concourse.mybir as mybir
import concourse.tile as tile
from concourse import bacc
from concourse.bass_utils import run_bass_kernel_spmd
from concourse.masks import make_identity

B, D, H, W, C = 4, 8, 32, 32, 512
NUM_HEADS = 8
HEAD_DIM = C // NUM_HEADS
SCALE = HEAD_DIM ** -0.5
N_TOK = B * D * H * W
N_CORES = 8
N_LOC = N_TOK // N_CORES          # 4096 own tokens per core
N_BATCH_TOK = 2 * N_LOC           # 8192 tokens per batch
N_CI = C // 128                   # 4 channel tiles
N_PAIRS = NUM_HEADS // 2
TT = 128
N_TTILES = N_BATCH_TOK // TT      # 64 token tiles for Gx
T_PER_CHUNK = 4                   # token tiles per DMA chunk
N_CHUNKS = N_TTILES // T_PER_CHUNK  # 16
N_OWN_TILES = N_LOC // TT         # 32 output token tiles

f32 = mybir.dt.float32
f32r = mybir.dt.float32r
bf16 = mybir.dt.bfloat16

N_KEEP_START = 12

_NC_CACHE = None


def build_nc():
    nc = bacc.Bacc(num_devices=N_CORES)

    xn = nc.declare_dram_parameter("xn", [N_BATCH_TOK, C], f32r, isOutput=False)
    xtb = nc.declare_dram_parameter("xtb", [C, N_LOC], bf16, isOutput=False)
    wk = nc.declare_dram_parameter("wk", [C, C], f32r, isOutput=False)
    wv = nc.declare_dram_parameter("wv", [C, C], f32r, isOutput=False)
    wqr = nc.declare_dram_parameter("wqr", [C, C], f32r, isOutput=False)
    wp = nc.declare_dram_parameter("wp", [C, C], f32r, isOutput=False)
    bp = nc.declare_dram_parameter("bp", [1, C], f32, isOutput=False)
    y = nc.declare_dram_parameter("y", [N_LOC, C], bf16, isOutput=True)

    xn_v = xn.rearrange("(t p) c -> p t c", p=128)      # [128, 64, 512]
    xtb_v = xtb.rearrange("(a p) n -> p a n", p=128)    # [128, 4, 4096]
    wk_v = wk.rearrange("(a p) f -> p a f", p=128)
    wv_v = wv.rearrange("(a p) f -> p a f", p=128)
    wqr_v = wqr.rearrange("(a p) f -> p a f", p=128)
    wp_v = wp.rearrange("(a p) f -> p a f", p=128)

    with tile.TileContext(nc) as tc, ExitStack() as ctx:
        const = ctx.enter_context(tc.tile_pool(name="const", bufs=1))
        persist = ctx.enter_context(tc.tile_pool(name="persist", bufs=1))
        sb = ctx.enter_context(tc.tile_pool(name="sb", bufs=2))

        wv_sb = const.tile([128, N_CI, C], f32r)
        nc.sync.dma_start(wv_sb[:], wv_v[:])
        wk_sb = const.tile([128, N_CI, C], f32r)
        nc.sync.dma_start(wk_sb[:], wk_v[:])

        ones_f32 = const.tile([1, 128], f32)
        nc.vector.memset(ones_f32[:], 1.0)
        ones_sb = const.tile([1, 128], f32r)
        nc.vector.tensor_copy(ones_sb[:], ones_f32[:])
        zrow_f32 = const.tile([1, 512], f32)
        nc.vector.memset(zrow_f32[:], 0.0)
        zrow_sb = const.tile([1, 512], f32r)
        nc.vector.tensor_copy(zrow_sb[:], zrow_f32[:])
        ident = const.tile([128, 128], f32)
        make_identity(nc, ident[:])

        with (
            tc.tile_pool(name="ps4", bufs=1, space="PSUM") as ps4,
            tc.tile_pool(name="ps2", bufs=1, space="PSUM") as ps2,
            tc.tile_pool(name="psy", bufs=2, space="PSUM") as psy,
        ):
            # HAM warm-up keepers during the initial DMA fill
            keep_ps = ps2.tile([128, C], f32, tag="k")
            for i in range(N_KEEP_START):
                nc.tensor.matmul(
                    keep_ps[:], ones_sb[:], zrow_sb[:],
                    start=(i == 0), stop=False, skip_group_check=True,
                )

            # ---------------- pass A: Gx = x^T x over the batch ------------
            gx_ps = ps4.tile([128, N_CI, C], f32, tag="big")
            for j in range(N_CHUNKS):
                xt = sb.tile([128, T_PER_CHUNK, C], f32r, tag="xt")
                nc.sync.dma_start(
                    xt[:], xn_v[:, j * T_PER_CHUNK:(j + 1) * T_PER_CHUNK, :]
                )
                for t in range(T_PER_CHUNK):
                    for ci in range(N_CI):
                        nc.tensor.matmul(
                            gx_ps[:, ci, :],
                            xt[:, t, ci * 128:(ci + 1) * 128],
                            xt[:, t, :],
                            start=(j == 0 and t == 0),
                            stop=(j == N_CHUNKS - 1 and t == T_PER_CHUNK - 1),
                            skip_group_check=True,
                        )

            # weights / x^T stream for the tail (drain after pass A's DMAs)
            wqr_sb = const.tile([128, N_CI, C], f32r)
            nc.sync.dma_start(wqr_sb[:], wqr_v[:])
            wp_sb = const.tile([128, N_CI, C], f32r)
            nc.sync.dma_start(wp_sb[:], wp_v[:])
            xtb_sb = const.tile([128, N_CI, N_LOC], bf16)
            nc.sync.dma_start(xtb_sb[:], xtb_v[:])
            bp_f32 = const.tile([128, C], f32)
            bp_bcast = bass.AP(
                tensor=bp[:].tensor,
                offset=0,
                ap=[[0, 128], [1, C]],
            )
            nc.sync.dma_start(bp_f32[:], bp_bcast)

            gx_sb = persist.tile([128, N_CI, C], f32r)
            nc.scalar.copy(gx_sb[:, 0:2, :], gx_ps[:, 0:2, :])
            nc.vector.tensor_copy(gx_sb[:, 2:4, :], gx_ps[:, 2:4, :])

            # ---------------- A = Gx @ Wv^T (Gx symmetric -> lhsT = Gx) ----
            a_ps = ps4.tile([128, N_CI, C], f32, tag="big")
            for j in range(N_CI):
                for a2 in range(N_CI):
                    nc.tensor.matmul(
                        a_ps[:, j, :],
                        gx_sb[:, a2, j * 128:(j + 1) * 128],
                        wv_sb[:, a2, :],
                        start=(a2 == 0), stop=(a2 == N_CI - 1),
                    )
            a_sb = persist.tile([128, N_CI, C], f32r)
            nc.scalar.copy(a_sb[:, 0:2, :], a_ps[:, 0:2, :])
            nc.vector.tensor_copy(a_sb[:, 2:4, :], a_ps[:, 2:4, :])

            # ---------------- Gram pairs (quad-blocked, F=256) -------------
            gram_ps = ps2.tile([128, N_PAIRS, 256], f32, tag="k")
            for p in range(N_PAIRS):
                q4 = p // 2
                for a2 in range(N_CI):
                    nc.tensor.matmul(
                        gram_ps[:, p, :],
                        wk_sb[:, a2, p * 128:(p + 1) * 128],
                        a_sb[:, a2, q4 * 256:(q4 + 1) * 256],
                        start=(a2 == 0), stop=(a2 == N_CI - 1),
                    )

            # pack 8 useful 64x64 blocks -> red_sb[d + 64*(h%2), h//2, :]
            red_sb = persist.tile([128, N_PAIRS, 64], f32)
            for h in range(NUM_HEADS):
                p = h // 2
                row0 = (h % 2) * 64
                col0 = (p % 2) * 128 + row0
                nc.vector.tensor_copy(
                    red_sb[row0:row0 + 64, h // 2, :],
                    gram_ps[row0:row0 + 64, p, col0:col0 + 64],
                )

            # ---- softmax over e on [128, pair, 64] ----
            nmax = sb.tile([128, N_PAIRS, 1], f32, tag="nmax")
            nc.vector.reduce_max(nmax[:], red_sb[:], axis=mybir.AxisListType.X, negate=True)
            shifted = sb.tile([128, N_PAIRS, 64], f32, tag="shifted")
            nc.vector.tensor_add(shifted[:], red_sb[:], nmax.broadcast_to([128, N_PAIRS, 64]))
            expd = sb.tile([128, N_PAIRS, 64], f32, tag="expd")
            nc.scalar.activation(expd[:], shifted[:], mybir.ActivationFunctionType.Exp)
            ssum = sb.tile([128, N_PAIRS, 1], f32, tag="ssum")
            nc.vector.reduce_sum(ssum[:], expd[:], axis=mybir.AxisListType.X)
            rsum = sb.tile([128, N_PAIRS, 1], f32, tag="rsum")
            nc.vector.reciprocal(rsum[:], ssum[:])
            probs = sb.tile([128, N_PAIRS, 64], f32, tag="probs")
            nc.vector.tensor_mul(probs[:], expd[:], rsum.broadcast_to([128, N_PAIRS, 64]))
            probs2 = sb.tile([64, NUM_HEADS, 64], f32, tag="probs2")
            nc.vector.tensor_copy(probs2[:, 0::2, :], probs[0:64, :, :])
            nc.vector.tensor_copy(probs2[:, 1::2, :], probs[64:128, :, :])
            zro = sb.tile([128, N_PAIRS, 128], f32, tag="zro")
            nc.vector.memset(zro[:], 0.0)
            atnT = persist.tile([128, N_PAIRS, 128], f32r)
            nc.vector.tensor_copy(atnT[:], zro[:])

            # ---- transpose probs -> block-diag pair lhsT (f32r) ----
            tr_ps = ps2.tile([64, NUM_HEADS, 64], f32, tag="k")
            for h in range(NUM_HEADS):
                nc.tensor.transpose(tr_ps[:, h, :], probs2[:, h, :], ident[0:64, 0:64])
            for h in range(NUM_HEADS):
                p = h // 2
                off = (h % 2) * 64
                nc.vector.tensor_copy(
                    atnT[off:off + 64, p, off:off + 64], tr_ps[:, h, :]
                )

            # ---------------- Wq_eff = BD @ Wq -----------------------------
            wqe_ps = ps4.tile([128, N_CI, C], f32, tag="big")
            for p in range(N_PAIRS):
                nc.tensor.matmul(
                    wqe_ps[:, p, :], atnT[:, p, :], wqr_sb[:, p, :],
                    start=True, stop=True,
                )
            wqe_sb = persist.tile([128, N_CI, C], f32r)
            nc.scalar.copy(wqe_sb[:, 0:2, :], wqe_ps[:, 0:2, :])
            nc.vector.tensor_copy(wqe_sb[:, 2:4, :], wqe_ps[:, 2:4, :])

            # ---------------- WeffT = Wq_eff^T @ Wproj^T -------------------
            wft_ps = ps4.tile([128, N_CI, C], f32, tag="big")
            for j in range(N_CI):
                for p in range(N_CI):
                    nc.tensor.matmul(
                        wft_ps[:, j, :],
                        wqe_sb[:, p, j * 128:(j + 1) * 128],
                        wp_sb[:, p, :],
                        start=(p == 0), stop=(p == N_CI - 1),
                    )
            wft_sb = persist.tile([128, N_CI, C], bf16)
            nc.scalar.copy(wft_sb[:, 0:2, :], wft_ps[:, 0:2, :])
            nc.vector.tensor_copy(wft_sb[:, 2:4, :], wft_ps[:, 2:4, :])

            # ---------------- final: y = x Weff^T + bias -------------------
            for t in range(N_OWN_TILES):
                y_ps = psy.tile([128, C], f32, tag="y")
                for ci in range(N_CI):
                    nc.tensor.matmul(
                        y_ps[:],
                        xtb_sb[:, ci, t * TT:(t + 1) * TT],
                        wft_sb[:, ci, :],
                        start=(ci == 0), stop=(ci == N_CI - 1),
                    )
                y_sb = sb.tile([128, C], bf16, tag="ysb", bufs=4)
                nc.vector.tensor_add(y_sb[:], y_ps[:], bp_f32[:])
                nc.sync.dma_start(y[t * TT:(t + 1) * TT, :], y_sb[:])

    nc.compile()
    return nc


def _get_nc():
    global _NC_CACHE
    if _NC_CACHE is None:
        _NC_CACHE = build_nc()
    return _NC_CACHE


def prep_inputs(x, Wqkv, Wproj, bproj):
    x = np.ascontiguousarray(np.asarray(x, dtype=np.float32))
    Wqkv = np.asarray(Wqkv, dtype=np.float32)
    Wproj = np.asarray(Wproj, dtype=np.float32)
    bproj = np.asarray(bproj, dtype=np.float32)

    xf = x.reshape(B, N_BATCH_TOK, C)
    wk_h = np.ascontiguousarray((Wqkv[C:2 * C] * np.float32(SCALE)).T)
    wv_h = np.ascontiguousarray(Wqkv[2 * C:3 * C].T)
    wqr_h = np.ascontiguousarray(Wqkv[0:C])
    wp_h = np.ascontiguousarray(Wproj.T)
    bp_h = np.ascontiguousarray(bproj.reshape(1, C))

    in_maps = []
    for i in range(N_CORES):
        b = i // 2
        t0 = (i % 2) * N_LOC
        own = xf[b, t0:t0 + N_LOC, :]
        pair = xf[b, N_LOC - t0:N_BATCH_TOK - t0, :]
        xn_l = np.ascontiguousarray(np.concatenate([own, pair], axis=0))
        xtb_l = np.ascontiguousarray(own.T).astype(ml_dtypes.bfloat16)
        in_maps.append({
            "xn": xn_l, "xtb": xtb_l,
            "wk": wk_h, "wv": wv_h, "wqr": wqr_h, "wp": wp_h, "bp": bp_h,
        })
    return in_maps


def gather_output(results):
    parts = [np.asarray(results[i]["y"]) for i in range(N_CORES)]
    out = np.concatenate(parts, axis=0).astype(np.float32)
    return out.reshape(B, D, H, W, C)


def kernel(x, Wqkv, Wproj, bproj, _trace=False, _tmpdir=None):
    nc = _get_nc()
    in_maps = prep_inputs(x, Wqkv, Wproj, bproj)
    res = run_bass_kernel_spmd(
        nc, in_maps, list(range(N_CORES)), trace=_trace, tmpdir=_tmpdir
    )
    out = gather_output(res.results)
    if _trace:
        kernel.last_exec_time_ns = res.exec_time_ns
        kernel.last_results = res
    return out


# revision 13
# speedup vs baseline: 1.9466x; 1.4249x over previous
"""ChannelAttention (B,D,H,W,C = 4,8,32,32,512; 8 heads, head_dim 64) on 8
Trainium2 NeuronCores, Bass/Tile SPMD. Fully data-parallel, zero cross-core
communication, with an algebraic restructuring that collapses most of the
FLOPs:

  attn_h = softmax(SCALE * k_h^T v_h) where k = x Wk^T, v = x Wv^T.
  k_h^T v_h = Wk_h (x^T x) Wv_h^T, so the per-head 64x64 Gram matrices only
  need Gx = x^T x (512x512 over the batch's 8192 tokens). The output
  y = ((x Wq^T) BD^T) Wproj^T + b (BD = blockdiag(attn_h)) then folds into a
  single GEMM y = x Weff^T + b with Weff = Wproj BD Wq (512x512, per batch).

Sharding: cores (2j, 2j+1) handle the two 4096-token halves of batch j. Each
core computes Gx over the full batch (duplicated k/v-side work, far cheaper
than any cross-core reduction), the tiny weight-space chain, then the final
GEMM for its own 4096 tokens.

Schedule per core:
  pass A : stream x token-major (16 chunks of [128,4,512]), accumulate
           Gx into PSUM (4 accumulation groups of F=512 fp32r matmuls).
           HAM keeper matmuls cover the initial DMA fill.
  chain  : Gx -> A = Gx Wv^T -> Gram pairs (quad-blocked, F=256) ->
           rowwise softmax over e -> PE-transpose probs -> block-diag pair
           lhsT -> Wq_eff = BD Wq -> WeffT = Wq_eff^T Wproj^T (-> bf16).
  final  : y = x Weff^T + bias for own tokens, lhsT from a host-supplied
           bf16 x^T stream (DMAed during pass A), out streamed as bf16.

Numerics: fp32r matmuls with fp32 PSUM accumulation through the Gram/softmax
chain; the final GEMM runs in bf16 (error there is linear, no softmax
amplification) and y returns as bf16 -> f32. End-to-end L2 relative error vs
the fp32 reference: ~3e-3 (threshold 2e-2).
"""

import numpy as np
from contextlib import ExitStack

import ml_dtypes

import concourse.bass as bass
import concourse.mybir as mybir
import concourse.tile as tile
from concourse import bacc
from concourse.bass_utils import run_bass_kernel_spmd
from concourse.masks import make_identity

B, D, H, W, C = 4, 8, 32, 32, 512
NUM_HEADS = 8
HEAD_DIM = C // NUM_HEADS
SCALE = HEAD_DIM ** -0.5
N_TOK = B * D * H * W
N_CORES = 8
N_LOC = N_TOK // N_CORES          # 4096 own tokens per core
N_BATCH_TOK = 2 * N_LOC           # 8192 tokens per batch
N_CI = C // 128                   # 4 channel tiles
N_PAIRS = NUM_HEADS // 2
TT = 128
N_TTILES = N_BATCH_TOK // TT      # 64 token tiles for Gx
T_PER_CHUNK = 4                   # token tiles per DMA chunk
N_CHUNKS = N_TTILES // T_PER_CHUNK  # 16
N_OWN_TILES = N_LOC // TT         # 32 output token tiles

f32 = mybir.dt.float32
f32r = mybir.dt.float32r
bf16 = mybir.dt.bfloat16

N_KEEP_START = 8

_NC_CACHE = None


def build_nc():
    nc = bacc.Bacc(num_devices=N_CORES)

    xn = nc.declare_dram_parameter("xn", [N_BATCH_TOK, C], f32r, isOutput=False)
    xtb = nc.declare_dram_parameter("xtb", [C, N_LOC], bf16, isOutput=False)
    wk = nc.declare_dram_parameter("wk", [C, C], f32r, isOutput=False)
    wv = nc.declare_dram_parameter("wv", [C, C], f32r, isOutput=False)
    wqr = nc.declare_dram_parameter("wqr", [C, C], f32r, isOutput=False)
    wp = nc.declare_dram_parameter("wp", [C, C], f32r, isOutput=False)
    bp = nc.declare_dram_parameter("bp", [1, C], f32, isOutput=False)
    y = nc.declare_dram_parameter("y", [N_LOC, C], bf16, isOutput=True)

    xn_v = xn.rearrange("(t p) c -> p t c", p=128)      # [128, 64, 512]
    xtb_v = xtb.rearrange("(a p) n -> p a n", p=128)    # [128, 4, 4096]
    wk_v = wk.rearrange("(a p) f -> p a f", p=128)
    wv_v = wv.rearrange("(a p) f -> p a f", p=128)
    wqr_v = wqr.rearrange("(a p) f -> p a f", p=128)
    wp_v = wp.rearrange("(a p) f -> p a f", p=128)

    with tile.TileContext(nc) as tc, ExitStack() as ctx:
        const = ctx.enter_context(tc.tile_pool(name="const", bufs=1))
        persist = ctx.enter_context(tc.tile_pool(name="persist", bufs=1))
        sb = ctx.enter_context(tc.tile_pool(name="sb", bufs=2))

        zblk_f32 = const.tile([128, 512], f32)
        nc.vector.memset(zblk_f32[:], 0.0)
        zblk_sb = const.tile([128, 512], f32r)
        nc.vector.tensor_copy(zblk_sb[:], zblk_f32[:])
        ident = const.tile([128, 128], f32)
        make_identity(nc, ident[:])
        ident_r = const.tile([128, 128], f32r)
        nc.vector.tensor_copy(ident_r[:], ident[:])

        with (
            tc.tile_pool(name="ps4", bufs=1, space="PSUM") as ps4,
            tc.tile_pool(name="ps2", bufs=1, space="PSUM") as ps2,
            tc.tile_pool(name="psy", bufs=2, space="PSUM") as psy,
        ):
            # ---------------- pass A: Gx = x^T x over the batch ------------
            # Symmetric trimming: row-block ci only computes columns
            # GX_COL0[ci]..512 (keeping every matmul F>=256 for fp32r full
            # rate); the 5 missing lower blocks are PE-transposed afterward.
            def load_chunk(j):
                xt = sb.tile([128, T_PER_CHUNK, C], f32r, tag="xt", bufs=4)
                nc.sync.dma_start(
                    xt[:], xn_v[:, j * T_PER_CHUNK:(j + 1) * T_PER_CHUNK, :]
                )
                return xt

            xt_tiles = {0: load_chunk(0), 1: load_chunk(1)}

            wv_sb = const.tile([128, N_CI, C], f32r)
            nc.sync.dma_start(wv_sb[:], wv_v[:])
            wk_sb = const.tile([128, N_CI, C], f32r)
            nc.sync.dma_start(wk_sb[:], wk_v[:])

            # real-work HAM keepers (K=128) covering the initial DMA fill
            keep_ps = ps2.tile([128, C], f32, tag="k")
            for i in range(N_KEEP_START):
                nc.tensor.matmul(
                    keep_ps[:], ident_r[:], zblk_sb[:],
                    start=(i == 0), stop=False, skip_group_check=True,
                )

            GX_COL0 = [0, 128, 256, 256]
            gx_ps = ps4.tile([128, N_CI, C], f32, tag="big")
            for j in range(N_CHUNKS):
                if j + 2 < N_CHUNKS:
                    xt_tiles[j + 2] = load_chunk(j + 2)
                xt = xt_tiles.pop(j)
                for t in range(T_PER_CHUNK):
                    for ci in range(N_CI):
                        c0 = GX_COL0[ci]
                        nc.tensor.matmul(
                            gx_ps[:, ci, c0:C],
                            xt[:, t, ci * 128:(ci + 1) * 128],
                            xt[:, t, c0:C],
                            start=(j == 0 and t == 0),
                            stop=(j == N_CHUNKS - 1 and t == T_PER_CHUNK - 1),
                            skip_group_check=True,
                        )

            # weights / x^T stream for the tail (drain after pass A's DMAs)
            wqr_sb = const.tile([128, N_CI, C], f32r)
            nc.sync.dma_start(wqr_sb[:], wqr_v[:])
            wp_sb = const.tile([128, N_CI, C], f32r)
            nc.sync.dma_start(wp_sb[:], wp_v[:])
            xtb_sb = const.tile([128, N_CI, N_LOC], bf16)
            nc.sync.dma_start(xtb_sb[:], xtb_v[:])
            bp_f32 = const.tile([128, C], f32)
            bp_bcast = bass.AP(
                tensor=bp[:].tensor,
                offset=0,
                ap=[[0, 128], [1, C]],
            )
            nc.sync.dma_start(bp_f32[:], bp_bcast)

            gx_sb = persist.tile([128, N_CI, C], f32r)
            nc.scalar.copy(gx_sb[:, 0, :], gx_ps[:, 0, :])
            nc.scalar.copy(gx_sb[:, 1, 128:C], gx_ps[:, 1, 128:C])
            nc.vector.tensor_copy(gx_sb[:, 2, 256:C], gx_ps[:, 2, 256:C])
            nc.vector.tensor_copy(gx_sb[:, 3, 256:C], gx_ps[:, 3, 256:C])

            # keep PE warm while the copies drain
            for i in range(3):
                nc.tensor.matmul(
                    keep_ps[:], ident_r[:], zblk_sb[:],
                    start=False, stop=False, skip_group_check=True,
                )

            # transpose fixups for the 5 missing lower Gx blocks
            FIX = [(1, 0), (2, 0), (2, 1), (3, 0), (3, 1)]
            gx_fix = persist.tile([128, 5, 128], f32r)
            tr1 = psy.tile([128, 3, 128], f32r, tag="y")
            tr2 = psy.tile([128, 2, 128], f32r, tag="y")
            for i, (r, cb) in enumerate(FIX):
                dst = tr1[:, i, :] if i < 3 else tr2[:, i - 3, :]
                nc.tensor.transpose(
                    dst, gx_sb[:, cb, r * 128:(r + 1) * 128], ident_r[:]
                )
            for i in range(5):
                src = tr1[:, i, :] if i < 3 else tr2[:, i - 3, :]
                if i % 2 == 0:
                    nc.scalar.copy(gx_fix[:, i, :], src)
                else:
                    nc.vector.tensor_copy(gx_fix[:, i, :], src)

            def gx_lhsT(a2, j):
                if j * 128 >= GX_COL0[a2]:
                    return gx_sb[:, a2, j * 128:(j + 1) * 128]
                return gx_fix[:, FIX.index((a2, j)), :]

            # ---------------- A = Gx @ Wv^T (Gx symmetric -> lhsT = Gx) ----
            a_ps = ps4.tile([128, N_CI, C], f32, tag="big")
            for j in [3, 2, 1, 0]:
                for a2 in range(N_CI):
                    nc.tensor.matmul(
                        a_ps[:, j, :],
                        gx_lhsT(a2, j),
                        wv_sb[:, a2, :],
                        start=(a2 == 0), stop=(a2 == N_CI - 1),
                    )
            a_sb = persist.tile([128, N_CI, C], f32r)
            nc.scalar.copy(a_sb[:, 0:2, :], a_ps[:, 0:2, :])
            nc.vector.tensor_copy(a_sb[:, 2:4, :], a_ps[:, 2:4, :])

            # ---------------- Gram pairs (quad-blocked, F=256) -------------
            gram_ps = ps2.tile([128, N_PAIRS, 256], f32, tag="k")
            for p in range(N_PAIRS):
                q4 = p // 2
                for a2 in range(N_CI):
                    nc.tensor.matmul(
                        gram_ps[:, p, :],
                        wk_sb[:, a2, p * 128:(p + 1) * 128],
                        a_sb[:, a2, q4 * 256:(q4 + 1) * 256],
                        start=(a2 == 0), stop=(a2 == N_CI - 1),
                    )

            # pack 8 useful 64x64 blocks -> red_sb[d + 64*(h%2), h//2, :]
            red_sb = persist.tile([128, N_PAIRS, 64], f32)
            for h in range(NUM_HEADS):
                p = h // 2
                row0 = (h % 2) * 64
                col0 = (p % 2) * 128 + row0
                nc.vector.tensor_copy(
                    red_sb[row0:row0 + 64, h // 2, :],
                    gram_ps[row0:row0 + 64, p, col0:col0 + 64],
                )

            # ---- softmax over e on [128, pair, 64] ----
            nmax = sb.tile([128, N_PAIRS, 1], f32, tag="nmax")
            nc.vector.reduce_max(nmax[:], red_sb[:], axis=mybir.AxisListType.X, negate=True)
            shifted = sb.tile([128, N_PAIRS, 64], f32, tag="shifted")
            nc.vector.tensor_add(shifted[:], red_sb[:], nmax.broadcast_to([128, N_PAIRS, 64]))
            expd = sb.tile([128, N_PAIRS, 64], f32, tag="expd")
            nc.scalar.activation(expd[:], shifted[:], mybir.ActivationFunctionType.Exp)
            ssum = sb.tile([128, N_PAIRS, 1], f32, tag="ssum")
            nc.vector.reduce_sum(ssum[:], expd[:], axis=mybir.AxisListType.X)
            rsum = sb.tile([128, N_PAIRS, 1], f32, tag="rsum")
            nc.vector.reciprocal(rsum[:], ssum[:])
            probs = sb.tile([128, N_PAIRS, 64], f32, tag="probs")
            nc.vector.tensor_mul(probs[:], expd[:], rsum.broadcast_to([128, N_PAIRS, 64]))
            probs2 = sb.tile([64, NUM_HEADS, 64], f32, tag="probs2")
            nc.vector.tensor_copy(probs2[:, 0::2, :], probs[0:64, :, :])
            nc.vector.tensor_copy(probs2[:, 1::2, :], probs[64:128, :, :])
            zro = sb.tile([128, N_PAIRS, 128], f32, tag="zro")
            nc.vector.memset(zro[:], 0.0)
            atnT = persist.tile([128, N_PAIRS, 128], f32r)
            nc.vector.tensor_copy(atnT[:], zro[:])

            # ---- transpose probs -> block-diag pair lhsT (f32r) ----
            tr_ps = ps2.tile([64, NUM_HEADS, 64], f32, tag="k")
            for h in range(NUM_HEADS):
                nc.tensor.transpose(tr_ps[:, h, :], probs2[:, h, :], ident[0:64, 0:64])
            for h in range(NUM_HEADS):
                p = h // 2
                off = (h % 2) * 64
                nc.vector.tensor_copy(
                    atnT[off:off + 64, p, off:off + 64], tr_ps[:, h, :]
                )

            # ---------------- Wq_eff = BD @ Wq -----------------------------
            wqe_ps = ps4.tile([128, N_CI, C], f32, tag="big")
            for p in range(N_PAIRS):
                nc.tensor.matmul(
                    wqe_ps[:, p, :], atnT[:, p, :], wqr_sb[:, p, :],
                    start=True, stop=True,
                )
            wqe_sb = persist.tile([128, N_CI, C], f32r)
            nc.scalar.copy(wqe_sb[:, 0:2, :], wqe_ps[:, 0:2, :])
            nc.vector.tensor_copy(wqe_sb[:, 2:4, :], wqe_ps[:, 2:4, :])

            # ---------------- WeffT = Wq_eff^T @ Wproj^T -------------------
            wft_ps = ps4.tile([128, N_CI, C], f32, tag="big")
            for j in range(N_CI):
                for p in range(N_CI):
                    nc.tensor.matmul(
                        wft_ps[:, j, :],
                        wqe_sb[:, p, j * 128:(j + 1) * 128],
                        wp_sb[:, p, :],
                        start=(p == 0), stop=(p == N_CI - 1),
                    )
            wft_sb = persist.tile([128, N_CI, C], bf16)
            nc.scalar.copy(wft_sb[:, 0:2, :], wft_ps[:, 0:2, :])
            nc.vector.tensor_copy(wft_sb[:, 2:4, :], wft_ps[:, 2:4, :])

            # ---------------- final: y = x Weff^T + bias -------------------
            for t in range(N_OWN_TILES):
                y_ps = psy.tile([128, C], f32, tag="y")
                for ci in range(N_CI):
                    nc.tensor.matmul(
                        y_ps[:],
                        xtb_sb[:, ci, t * TT:(t + 1) * TT],
                        wft_sb[:, ci, :],
                        start=(ci == 0), stop=(ci == N_CI - 1),
                    )
                y_sb = sb.tile([128, C], bf16, tag="ysb", bufs=4)
                nc.vector.tensor_add(y_sb[:], y_ps[:], bp_f32[:])
                nc.sync.dma_start(y[t * TT:(t + 1) * TT, :], y_sb[:])

    nc.compile()
    return nc


def _get_nc():
    global _NC_CACHE
    if _NC_CACHE is None:
        _NC_CACHE = build_nc()
    return _NC_CACHE


def prep_inputs(x, Wqkv, Wproj, bproj):
    x = np.ascontiguousarray(np.asarray(x, dtype=np.float32))
    Wqkv = np.asarray(Wqkv, dtype=np.float32)
    Wproj = np.asarray(Wproj, dtype=np.float32)
    bproj = np.asarray(bproj, dtype=np.float32)

    xf = x.reshape(B, N_BATCH_TOK, C)
    wk_h = np.ascontiguousarray((Wqkv[C:2 * C] * np.float32(SCALE)).T)
    wv_h = np.ascontiguousarray(Wqkv[2 * C:3 * C].T)
    wqr_h = np.ascontiguousarray(Wqkv[0:C])
    wp_h = np.ascontiguousarray(Wproj.T)
    bp_h = np.ascontiguousarray(bproj.reshape(1, C))

    in_maps = []
    for i in range(N_CORES):
        b = i // 2
        t0 = (i % 2) * N_LOC
        own = xf[b, t0:t0 + N_LOC, :]
        pair = xf[b, N_LOC - t0:N_BATCH_TOK - t0, :]
        xn_l = np.ascontiguousarray(np.concatenate([own, pair], axis=0))
        xtb_l = np.ascontiguousarray(own.T).astype(ml_dtypes.bfloat16)
        in_maps.append({
            "xn": xn_l, "xtb": xtb_l,
            "wk": wk_h, "wv": wv_h, "wqr": wqr_h, "wp": wp_h, "bp": bp_h,
        })
    return in_maps


def gather_output(results):
    parts = [np.asarray(results[i]["y"]) for i in range(N_CORES)]
    out = np.concatenate(parts, axis=0).astype(np.float32)
    return out.reshape(B, D, H, W, C)


def kernel(x, Wqkv, Wproj, bproj, _trace=False, _tmpdir=None):
    nc = _get_nc()
    in_maps = prep_inputs(x, Wqkv, Wproj, bproj)
    res = run_bass_kernel_spmd(
        nc, in_maps, list(range(N_CORES)), trace=_trace, tmpdir=_tmpdir
    )
    out = gather_output(res.results)
    if _trace:
        kernel.last_exec_time_ns = res.exec_time_ns
        kernel.last_results = res
    return out
